# revision 26
# baseline (speedup 1.0000x reference)
import numpy as np
from contextlib import ExitStack

import concourse.bass as bass
import concourse.tile as tile
from concourse import mybir, bass_utils
from concourse.masks import make_identity

N, E, D, EF = 50000, 800000, 128, 64
NH, DH = 8, 16
NCORES = 8
NPC = N // NCORES           # 6250 nodes per core
W = 49                      # windows of 128 nodes per core
NPAD = W * 128              # 6272
EPS = 1e-5
EXP_BIAS = -2.7726          # exp scaled by 2^-4; cancels in ws/den ratio

F32 = mybir.dt.float32
F16 = mybir.dt.float16
AF = mybir.ActivationFunctionType
ALU = mybir.AluOpType
AX = mybir.AxisListType


def _preprocess(inputs):
    bf = np.float16
    h = np.ascontiguousarray(inputs['h'], np.float32)
    ef = np.asarray(inputs['edge_feat'], np.float32)
    e_w = np.asarray(inputs['e_w'], np.float32)
    src = np.asarray(inputs['edge_index'][0], np.int64)
    dst = np.asarray(inputs['edge_index'][1], np.int64)

    order = np.argsort(dst, kind='stable')
    src_s, dst_s = src[order], dst[order]
    ew_s, ef_s = e_w[order], ef[order]

    bounds = np.searchsorted(dst_s, np.arange(NCORES + 1) * NPC)
    cnt = np.zeros((NCORES, W), np.int64)
    pc = []
    for c in range(NCORES):
        lo, hi = int(bounds[c]), int(bounds[c + 1])
        dl = dst_s[lo:hi] - c * NPC
        cnt[c] = np.bincount(dl >> 7, minlength=W)
        pc.append((lo, dl))
    T = np.maximum(1, (cnt.max(axis=0) + 127) // 128).astype(np.int64)
    base_t = np.zeros(W + 1, np.int64)
    base_t[1:] = np.cumsum(T)
    Ttot = int(base_t[-1])
    EPAD = Ttot * 128

    g32 = lambda x: np.asarray(x, np.float32)
    tobf = lambda x: np.ascontiguousarray(np.asarray(x, np.float32).astype(bf))
    # fused first-layer weights (k|v) and their per-group column sums
    w1cat = np.concatenate([g32(inputs['hk_W1']), g32(inputs['hv_W1'])], axis=1)
    w1r = w1cat.astype(bf).astype(np.float32)   # bf16-rounded, for exact-sum cols
    gsum = lambda wb: np.stack([wb[:, 0:128].sum(1), wb[:, 128:256].sum(1)], axis=1)
    wq1r = g32(inputs['hq_W1']).astype(bf).astype(np.float32)
    wn1r = g32(inputs['no_W1']).astype(bf).astype(np.float32)
    shared = dict(
        w1_ef=tobf(w1cat[0:EF]), w1_hi=tobf(w1cat[EF:EF + 128]),
        w1_hj=tobf(w1cat[EF + 128:EF + 256]),
        w1s_ef=tobf(gsum(w1r[0:EF])), w1s_hi=tobf(gsum(w1r[EF:EF + 128])),
        w1s_hj=tobf(gsum(w1r[EF + 128:EF + 256])),
        w2k=tobf(inputs['hk_W2']), w2v=tobf(inputs['hv_W2']),
        wq1=tobf(inputs['hq_W1']), wq2=tobf(inputs['hq_W2']),
        wq1s=tobf(wq1r.sum(1, keepdims=True)),
        wn1a=tobf(g32(inputs['no_W1'])[0:128]),
        wn1h=tobf(g32(inputs['no_W1'])[128:256]),
        wn1s_a=tobf(wn1r[0:128].sum(1, keepdims=True)),
        wn1s_h=tobf(wn1r[128:256].sum(1, keepdims=True)),
        wn2=tobf(inputs['no_W2']),
    )
    hb = h.astype(bf)
    # transposed node table with a zero column at index N (padding target)
    hTz = np.concatenate([np.ascontiguousarray(hb.T),
                          np.zeros((D, 1), bf)], axis=1)

    in_maps = []
    for c in range(NCORES):
        lo, dl = pc[c]
        ws = np.zeros(W + 1, np.int64)
        ws[1:] = np.cumsum(cnt[c])
        srcg = np.full(EPAD, N, np.int64)
        dstg = np.full(EPAD, N, np.int64)
        dstr = np.full(EPAD, 999, np.int64)
        eww = np.zeros(EPAD, np.float32)
        eftp = np.zeros((EPAD, EF), np.float32)
        for w in range(W):
            a, b = int(ws[w]), int(ws[w + 1])
            n = b - a
            o = int(base_t[w]) * 128
            srcg[o:o + n] = src_s[lo + a:lo + b]
            dstg[o:o + n] = dst_s[lo + a:lo + b]
            dstr[o:o + n] = dl[a:b] - (w << 7)
            eww[o:o + n] = ew_s[lo + a:lo + b]
            eftp[o:o + n] = ef_s[lo + a:lo + b]
        hiT = hTz[:, dstg]          # [128, EPAD] bf16
        hjT = hTz[:, srcg]
        ohne = (np.arange(128, dtype=np.int64)[:, None]
                == dstr[None, :]).astype(bf)                 # [128, EPAD]
        d2 = dstr.reshape(Ttot, 128)
        ohen = (d2[:, :, None]
                == np.arange(128, dtype=np.int64)[None, None, :]).astype(np.float16)
        ohen = np.ascontiguousarray(ohen.transpose(1, 0, 2))  # [128e, Ttot, 128n]

        combo = np.empty((128, Ttot, 512), np.uint16)
        combo[:, :, 0:128] = hiT.view(np.uint16).reshape(128, Ttot, 128)
        combo[:, :, 128:256] = hjT.view(np.uint16).reshape(128, Ttot, 128)
        combo[:, :, 256:384] = ohne.view(np.uint16).reshape(128, Ttot, 128)
        combo[:, :, 384:512] = ohen.view(np.uint16)

        hT_own = np.zeros((D, NPAD), bf)
        hT_own[:, :NPC] = hTz[:, c * NPC:(c + 1) * NPC]
        m = dict(shared)
        m.update(
            combo=combo.reshape(128, Ttot * 512).view(bf),
            eftW=np.ascontiguousarray(eftp.T).astype(bf),
            ew2=np.ascontiguousarray(eww.reshape(Ttot, 128).T),
            hT_own=hT_own,
        )
        in_maps.append(m)
    return in_maps, [int(x) for x in T], [int(x) for x in base_t]


def _build(T, base_t):
    Ttot = base_t[-1]
    Tmax = max(T)
    nc = bass.Bass(target_bir_lowering=False, debug=False)
    dt = nc.dram_tensor
    combo_d = dt('combo', [128, Ttot * 512], F16, kind='ExternalInput')
    eft_d = dt('eftW', [EF, Ttot * 128], F16, kind='ExternalInput')
    ew_d = dt('ew2', [128, Ttot], F32, kind='ExternalInput')
    hT_d = dt('hT_own', [D, NPAD], F16, kind='ExternalInput')
    wd = {}
    for nm, p, q in [('w1_ef', EF, 256), ('w1_hi', 128, 256), ('w1_hj', 128, 256),
                     ('w1s_ef', EF, 2), ('w1s_hi', 128, 2), ('w1s_hj', 128, 2),
                     ('w2k', 128, 128), ('w2v', 128, 128),
                     ('wq1', 128, 128), ('wq1s', 128, 1), ('wq2', 128, 128),
                     ('wn1a', 128, 128), ('wn1h', 128, 128),
                     ('wn1s_a', 128, 1), ('wn1s_h', 128, 1), ('wn2', 128, 128)]:
        wd[nm] = dt(nm, [p, q], F16, kind='ExternalInput')
    out_d = dt('out', [NPAD, D], F32, kind='ExternalOutput')

    with ExitStack() as ctx:
        tc = ctx.enter_context(tile.TileContext(nc))
        cp = ctx.enter_context(tc.tile_pool(name='consts', bufs=1))
        wp = ctx.enter_context(tc.tile_pool(name='win', bufs=3))
        tp = ctx.enter_context(tc.tile_pool(name='mac', bufs=8))
        pb1 = ctx.enter_context(tc.tile_pool(name='pA', bufs=2, space='PSUM'))
        pb2 = ctx.enter_context(tc.tile_pool(name='pB', bufs=1, space='PSUM'))
        pb3 = ctx.enter_context(tc.tile_pool(name='pC', bufs=2, space='PSUM'))
        pac = ctx.enter_context(tc.tile_pool(name='pD', bufs=2, space='PSUM'))
        pst = ctx.enter_context(tc.tile_pool(name='pE', bufs=1, space='PSUM'))

        ident = cp.tile([128, 128], F16, name='ident')
        make_identity(nc, ident[:])
        eps_col = cp.tile([128, 1], F32, name='eps_col')
        nc.gpsimd.memset(eps_col[:], float(EPS))
        ebias_col = cp.tile([128, 1], F32, name='ebias_col')
        nc.gpsimd.memset(ebias_col[:], float(EXP_BIAS))
        wsb = {}
        for nm, dr in wd.items():
            t = cp.tile(list(dr.shape), F16, name=nm + '_s')
            nc.sync.dma_start(out=t[:], in_=dr[:])
            wsb[nm] = t

        def ln_small(ps_ap, sum_ap, tag):
            # LayerNorm stats for one [128,128] group. Returns (rstd, nmr) cols.
            st = wp.tile([128, 8], F32, name='stw' + tag)
            sq = wp.tile([128, 128], F16, name='sqw' + tag)
            nc.scalar.activation(sq[:], ps_ap, AF.Square)
            with nc.allow_low_precision(reason='ln sumsq'):
                nc.vector.reduce_sum(out=st[:, 1:2], in_=sq[:], axis=AX.X)
            nc.vector.tensor_scalar(st[:, 0:1], sum_ap, -1.0 / 128, None, op0=ALU.mult)
            nc.vector.tensor_scalar(st[:, 2:3], st[:, 1:2], 1.0 / 128, None, op0=ALU.mult)
            nc.gpsimd.tensor_tensor(st[:, 3:4], st[:, 0:1], st[:, 0:1], op=ALU.mult)
            nc.gpsimd.tensor_tensor(st[:, 4:5], st[:, 2:3], st[:, 3:4], op=ALU.subtract)
            nc.scalar.activation(st[:, 5:6], st[:, 4:5], AF.Ln, bias=eps_col[:])
            nc.scalar.activation(st[:, 6:7], st[:, 5:6], AF.Exp, scale=-0.5)
            nc.gpsimd.tensor_tensor(st[:, 7:8], st[:, 0:1], st[:, 6:7], op=ALU.mult)
            return st[:, 6:7], st[:, 7:8]

        win_t = {}

        def emit_dma(w):
            Tw, tb = T[w], base_t[w]
            combo_w = wp.tile([128, Tmax * 512], F16, name='combo_w')
            nc.sync.dma_start(out=combo_w[:, 0:Tw * 512],
                              in_=combo_d[:, tb * 512:(tb + Tw) * 512])
            eft_w = wp.tile([EF, Tmax * 128], F16, name='eft_w')
            nc.sync.dma_start(out=eft_w[:, 0:Tw * 128],
                              in_=eft_d[:, tb * 128:(tb + Tw) * 128])
            ew_w = wp.tile([128, Tmax], F32, name='ew_w')
            nc.sync.dma_start(out=ew_w[:, 0:Tw], in_=ew_d[:, tb:tb + Tw])
            hT_w = wp.tile([128, 128], F16, name='hT_w')
            nc.sync.dma_start(out=hT_w[:], in_=hT_d[:, w * 128:(w + 1) * 128])
            win_t[w] = (combo_w, eft_w, ew_w, hT_w)

        pend_epi = [None]
        emit_dma(0)
        for w in range(W):
            Tw, tb = T[w], base_t[w]
            combo_w, eft_w, ew_w, hT_w = win_t.pop(w)

            accst = pac.tile([128, 512], F32, name='accst')
            stt = pst.tile([128, 512], F32, name='stt')

            # ---- q = MLP_hq(h_win)
            Aq = pb2.tile([128, 256], F32, name='qe')
            nc.tensor.matmul(Aq[:, 0:128], hT_w[:], wsb['wq1'][:], start=True, stop=True)
            nc.tensor.matmul(accst[:, 156:157], hT_w[:], wsb['wq1s'][:], start=True, stop=True)
            qr, qn = ln_small(Aq[:, 0:128], accst[:, 156:157], 'q')
            qrelu = wp.tile([128, 128], F16, name='qrelu')
            nc.scalar.activation(qrelu[:], Aq[:, 0:128], AF.Relu, scale=qr, bias=qn)
            nc.tensor.transpose(Aq[:, 0:64].bitcast(F16), qrelu[:], ident[:])
            qrT = wp.tile([128, 128], F16, name='qrT')
            nc.scalar.activation(qrT[:], Aq[:, 0:64].bitcast(F16), AF.Copy)
            nc.tensor.matmul(Aq[:, 128:256], qrT[:], wsb['wq2'][:], start=True, stop=True)
            q_sb = wp.tile([128, 128], F16, name='q_sb')
            nc.scalar.activation(q_sb[:], Aq[:, 128:256], AF.Copy)

            nmac = (Tw + 1) // 2
            MS = {}

            def S1(m):
                # PE: first-layer matmuls + LN-sum columns
                tl = [m * 2 + i for i in range(2) if m * 2 + i < Tw]
                s = MS[m] = dict(tl=tl, k=len(tl), G=2 * len(tl),
                                 scol=4 * (m % 3))
                hdn2 = s['hdn2'] = pb1.tile([128, 512], F32, name='hdn2')
                for i, t in enumerate(tl):
                    cw = 512 * t
                    hi_ap = combo_w[:, cw:cw + 128]
                    hj_ap = combo_w[:, cw + 128:cw + 256]
                    ef_ap = eft_w[:, 128 * t:128 * t + 128]
                    dc = 256 * i
                    nc.tensor.matmul(hdn2[:, dc:dc + 256], hj_ap, wsb['w1_hj'][:],
                                     start=True, stop=False)
                    nc.tensor.matmul(hdn2[:, dc:dc + 256], hi_ap, wsb['w1_hi'][:],
                                     start=False, stop=False)
                    nc.tensor.matmul(hdn2[:, dc:dc + 256], ef_ap, wsb['w1_ef'][:],
                                     start=False, stop=True)
                    sc = s['scol'] + 2 * i
                    nc.tensor.matmul(stt[:, sc:sc + 2], hj_ap, wsb['w1s_hj'][:],
                                     start=True, stop=False)
                    nc.tensor.matmul(stt[:, sc:sc + 2], hi_ap, wsb['w1s_hi'][:],
                                     start=False, stop=False)
                    nc.tensor.matmul(stt[:, sc:sc + 2], ef_ap, wsb['w1s_ef'][:],
                                     start=False, stop=True)

            def S2(m):
                # vector: LN stats chain + squares + relus
                s = MS[m]
                tl, k, G, scol = s['tl'], s['k'], s['G'], s['scol']
                hdn2 = s['hdn2']
                st = s['st'] = tp.tile([128, 24], F32, name='st')
                q2b = tp.tile([128, 4], F32, name='q2b')
                rst = s['rst'] = tp.tile([128, 4], F32, name='rst')
                sq2 = tp.tile([128, 512], F16, name='sq2')
                nc.vector.tensor_scalar(st[:, 0:G], stt[:, scol:scol + G],
                                        -1.0 / 128, None, op0=ALU.mult)
                nc.scalar.activation(sq2[:, 0:256 * k], hdn2[:, 0:256 * k], AF.Square)
                with nc.allow_low_precision(reason='ln sumsq'):
                    nc.vector.reduce_sum(
                        out=q2b[:, 0:G],
                        in_=sq2[:, 0:256 * k].rearrange('p (g f) -> p g f', g=G),
                        axis=AX.X)
                nc.vector.tensor_scalar(st[:, 4:4 + G], q2b[:, 0:G], 1.0 / 128,
                                        None, op0=ALU.mult)
                nc.gpsimd.tensor_tensor(st[:, 8:8 + G], st[:, 0:G], st[:, 0:G],
                                        op=ALU.mult)
                nc.gpsimd.tensor_tensor(st[:, 12:12 + G], st[:, 4:4 + G],
                                        st[:, 8:8 + G], op=ALU.subtract)
                nc.scalar.activation(st[:, 16:16 + G], st[:, 12:12 + G], AF.Ln,
                                     bias=eps_col[:])
                nc.scalar.activation(rst[:, 0:G], st[:, 16:16 + G], AF.Exp, scale=-0.5)
                rs25 = s['rs25'] = tp.tile([128, 4], F32, name='rs25')
                nc.vector.tensor_scalar(rs25[:, 0:G], rst[:, 0:G], 0.25, None,
                                        op0=ALU.mult)
                relu1 = s['relu1'] = tp.tile([128, 512], F16, name='relu1')
                for i, t in enumerate(tl):
                    dc = 256 * i
                    nc.scalar.activation(relu1[:, dc:dc + 128], hdn2[:, dc:dc + 128],
                                         AF.Relu, bias=st[:, 2 * i:2 * i + 1])
                    nc.vector.tensor_scalar(relu1[:, dc + 128:dc + 256],
                                            hdn2[:, dc + 128:dc + 256],
                                            st[:, 2 * i + 1:2 * i + 2], 0.0,
                                            op0=ALU.add, op1=ALU.max)

            def S3a(m):
                # PE: transposes + qe gather; vector: PSUM->SBUF copies
                s = MS[m]
                tl, k = s['tl'], s['k']
                relu1 = s['relu1']
                hdn2 = s['hdn2']
                qet = s['qet'] = pb2.tile([128, 256], F32, name='qe')
                for i, t in enumerate(tl):
                    dc = 256 * i
                    nc.tensor.transpose(
                        hdn2[:, 64 * i:64 * i + 64].bitcast(F16),
                        relu1[:, dc:dc + 128], ident[:])
                    nc.tensor.transpose(
                        hdn2[:, 128 + 64 * i:192 + 64 * i].bitcast(F16),
                        relu1[:, dc + 128:dc + 256], ident[:])
                    on_ap = combo_w[:, 512 * t + 256:512 * t + 384]
                    nc.tensor.matmul(qet[:, 128 * i:128 * i + 128], on_ap, q_sb[:],
                                     start=True, stop=True)
                kT2 = s['kT2'] = tp.tile([128, 256], F16, name='kT2')
                vT2 = s['vT2'] = tp.tile([128, 256], F16, name='vT2')
                nc.scalar.activation(kT2[:, 0:128 * k],
                                     hdn2[:, 0:64 * k].bitcast(F16), AF.Copy)
                nc.vector.tensor_copy(vT2[:, 0:128 * k],
                                      hdn2[:, 128:128 + 64 * k].bitcast(F16))

            def S3b(m):
                # PE: second-layer matmuls
                s = MS[m]
                kT2, vT2 = s['kT2'], s['vT2']
                kv2 = s['kv2'] = pb3.tile([128, 512], F32, name='kv2')
                for i, t in enumerate(s['tl']):
                    dc = 256 * i
                    nc.tensor.matmul(kv2[:, dc:dc + 128], kT2[:, 128 * i:128 * i + 128],
                                     wsb['w2k'][:], start=True, stop=True)
                    nc.tensor.matmul(kv2[:, dc + 128:dc + 256],
                                     vT2[:, 128 * i:128 * i + 128],
                                     wsb['w2v'][:], start=True, stop=True)

            def S4(m):
                # vector: logits, exp, attention-weighted values
                s = MS[m]
                tl, k, rst = s['tl'], s['k'], s['rst']
                qet, kv2 = s['qet'], s['kv2']
                rs25 = s['rs25']
                qe_sb = tp.tile([128, 256], F16, name='qe_sb')
                for i, t in enumerate(tl):
                    nc.scalar.activation(qe_sb[:, 128 * i:128 * i + 128],
                                         qet[:, 128 * i:128 * i + 128], AF.Copy,
                                         scale=rs25[:, 2 * i:2 * i + 1])
                qk2 = tp.tile([128, 256], F16, name='qk2')
                for i, t in enumerate(tl):
                    nc.vector.tensor_tensor(qk2[:, 128 * i:128 * i + 128],
                                            qe_sb[:, 128 * i:128 * i + 128],
                                            kv2[:, 256 * i:256 * i + 128],
                                            op=ALU.mult)
                lg2 = tp.tile([128, 16], F32, name='lg2')
                with nc.allow_low_precision(reason='logit sums'):
                    nc.vector.reduce_sum(
                        out=lg2[:, 0:NH * k],
                        in_=qk2[:, 0:128 * k].rearrange('p (g f) -> p g f', g=NH * k),
                        axis=AX.X)
                ex2 = s['ex2'] = tp.tile([128, 16], F16, name='ex2')
                nc.scalar.activation(ex2[:, 0:NH * k], lg2[:, 0:NH * k], AF.Exp,
                                     bias=ebias_col[:])
                ewr = tp.tile([128, 2], F32, name='ewr')
                exv = tp.tile([128, 16], F32, name='exv')
                X2 = s['X2'] = tp.tile([128, 256], F16, name='X2')
                for i, t in enumerate(tl):
                    nc.gpsimd.tensor_tensor(ewr[:, i:i + 1], ew_w[:, t:t + 1],
                                            rst[:, 2 * i + 1:2 * i + 2], op=ALU.mult)
                    nc.gpsimd.tensor_scalar(exv[:, 8 * i:8 * i + 8],
                                            ex2[:, 8 * i:8 * i + 8],
                                            ewr[:, i:i + 1], None, op0=ALU.mult)
                    nc.vector.tensor_tensor(
                        X2[:, 128 * i:128 * i + 128].rearrange('p (g f) -> p g f', g=NH),
                        kv2[:, 256 * i + 128:256 * i + 256].rearrange('p (g f) -> p g f', g=NH),
                        exv[:, 8 * i:8 * i + 8].to_broadcast([128, NH, DH]),
                        op=ALU.mult)

            def S5(m):
                # PE: scatter-accumulate weighted values + exp sums.
                # One accumulation group per window (PSUM zero-regions are
                # bank-wide): start only on the first matmul, stop on the last.
                s = MS[m]
                for i, t in enumerate(s['tl']):
                    oe_ap = combo_w[:, 512 * t + 384:512 * t + 512]
                    nc.tensor.matmul(accst[:, 0:128], oe_ap,
                                     s['X2'][:, 128 * i:128 * i + 128],
                                     start=(t == 0), stop=False)
                    nc.tensor.matmul(accst[:, 128:136], oe_ap,
                                     s['ex2'][:, 8 * i:8 * i + 8],
                                     start=False, stop=(t == Tw - 1))

            for r in range(nmac + 4):
                if 0 <= r - 1 < nmac:
                    S2(r - 1)
                if 0 <= r - 2 < nmac:
                    S3a(r - 2)
                if 0 <= r - 2 < nmac:
                    S3b(r - 2)
                if 0 <= r - 3 < nmac:
                    S4(r - 3)
                if 0 <= r - 4 < nmac:
                    S5(r - 4)
                if r < nmac:
                    S1(r)
                if r == 0 and w + 1 < W:
                    emit_dma(w + 1)
                if r == 1 and pend_epi[0] is not None:
                    pend_epi[0]()
                    pend_epi[0] = None

            def make_epi(accst=accst, hT_w=hT_w, w=w):
                def epi():
                    # attn / denominator / MLP_no, in the accst bank's scratch
                    den = wp.tile([128, NH], F32, name='den')
                    nc.vector.tensor_scalar(den[:], accst[:, 128:136], 1e-30,
                                            None, op0=ALU.max)
                    rden = wp.tile([128, NH], F32, name='rden')
                    nc.vector.reciprocal(rden[:], den[:])
                    attn = wp.tile([128, 128], F16, name='attn')
                    nc.vector.tensor_tensor(
                        attn[:].rearrange('p (g f) -> p g f', g=NH),
                        accst[:, 0:128].rearrange('p (g f) -> p g f', g=NH),
                        rden[:].to_broadcast([128, NH, DH]), op=ALU.mult)
                    nc.tensor.transpose(accst[:, 160:224].bitcast(F16), attn[:],
                                        ident[:])
                    attnT = wp.tile([128, 128], F16, name='attnT')
                    nc.scalar.activation(attnT[:], accst[:, 160:224].bitcast(F16),
                                         AF.Copy)
                    nc.tensor.matmul(accst[:, 224:352], attnT[:], wsb['wn1a'][:],
                                     start=True, stop=False)
                    nc.tensor.matmul(accst[:, 224:352], hT_w[:], wsb['wn1h'][:],
                                     start=False, stop=True)
                    nc.tensor.matmul(accst[:, 157:158], attnT[:], wsb['wn1s_a'][:],
                                     start=True, stop=False)
                    nc.tensor.matmul(accst[:, 157:158], hT_w[:], wsb['wn1s_h'][:],
                                     start=False, stop=True)
                    nr, nn_ = ln_small(accst[:, 224:352], accst[:, 157:158], 'n')
                    norelu = wp.tile([128, 128], F16, name='norelu')
                    nc.scalar.activation(norelu[:], accst[:, 224:352], AF.Relu,
                                         scale=nr, bias=nn_)
                    nc.tensor.transpose(accst[:, 160:224].bitcast(F16), norelu[:],
                                        ident[:])
                    norT = wp.tile([128, 128], F16, name='norT')
                    nc.scalar.activation(norT[:], accst[:, 160:224].bitcast(F16),
                                         AF.Copy)
                    nc.tensor.matmul(accst[:, 224:352], norT[:], wsb['wn2'][:],
                                     start=True, stop=True)
                    out_sb = wp.tile([128, 128], F32, name='out_sb')
                    nc.scalar.activation(out_sb[:], accst[:, 224:352], AF.Copy)
                    nc.sync.dma_start(out=out_d[w * 128:(w + 1) * 128, :],
                                      in_=out_sb[:])
                return epi
            pend_epi[0] = make_epi()
        pend_epi[0]()
    return nc


def kernel(_trace=False, **inputs):
    import bass_rust
    in_maps, T, base_t = _preprocess(inputs)
    nc = _build(T, base_t)
    bass_rust.generate_event_semaphores(nc)
    res = bass_utils.run_bass_kernel_spmd(nc, in_maps, core_ids=list(range(NCORES)),
                                          trace=_trace)
    out = np.concatenate(
        [np.asarray(res.results[c]['out'])[:NPC] for c in range(NCORES)], axis=0)
    if _trace:
        return out.astype(np.float32), res
    return out.astype(np.float32)
